# revision 51
# baseline (speedup 1.0000x reference)
"""Trainium2 Bass kernel: SNN Leaky-Integrate-and-Fire layer.

Computes, for x [T=1024, N_IN=4096] f32 and W [N_OUT=4096, N_IN=4096] f32:
    cur = x @ W.T                                   # [T, N_OUT]
    mem_t = 0.9*mem_{t-1} + cur_t - (mem_{t-1} > 1)  # scan over T
    spk_t = (mem_t > 1)
returning (spk_rec, mem_rec), both [T, N_OUT] f32.

Sharding: N_OUT split across 8 NeuronCores (512 neurons each); x is
replicated. No cross-core communication.

Default device algorithm per core ("fp16512", neurons on partitions,
time along the free dim):
  - single-pass fp16 matmul: x and W are cast to fp16 host-side
    (products are exact in the fp32 PSUM accumulate; quantization gives
    ~508/4.2M spike flips, rel err 1.7e-2 < 2e-2). 256 matmuls:
    K=4096 in 32 tiles x O=512 in 4 o-tiles x T=1024 in 2 chunks of 512,
    each (o, chunk) accumulating into its own PSUM bank.
  - the full nonlinear LIF scan runs in ONE custom DVE instruction per
    (o, chunk) [LIF_SCAN_ANT, 3 cycles/element, see _build_lif_uops_v2],
    chaining membrane state across chunks via a per-partition scalar AP.
  - mem is DMA'd out in f32; spikes are computed host-side as mem > 1
    (bit-identical to an on-device is_gt on the same f32 values).
Older variants kept for comparison: "split3" (3-pass bf16 hi/lo matmul +
A/B-decomposed scan, 2 serial DVE instrs per timestep) and "custom"
(split3 matmul + custom DVE scan).
"""

import numpy as np

T = 1024
N_IN = 4096
N_OUT = 4096
N_CORES = 8
O_SHARD = N_OUT // N_CORES  # 512
KT = N_IN // 128  # 32 k-tiles
OT = O_SHARD // 128  # 4 o-tiles
BETA = 0.9
THRESHOLD = 1.0

_CACHE = {}

LIF_OP_NAME = "LIF_SCAN_ANT"


def _build_lif_uops_v2():
    """Hand-written DVE uop program for the LIF recurrence:

        m' = beta*m + c[i] - (m > 1)    (beta = CONST_0/s0, init m = CONST_1/s1)
        out[i] = m'   (out len == src len, 3 cycles per element)

    State m lives in block1's out-flop (+ a-flop for the block0 backward
    read). Per element e with P0/P1/P2 entering at consecutive cycles:
      P0: b1: u = beta*m        (CURR_ALU_OUT x PREV_DELAY_1)
      P1: b0: r = IS_GT(m, 1)   (NEXT_ALU_OUT_A vs PREV_DELAY_2) - reads the
          a-flop one cycle after P2(e-1) wrote m', so it is fresh
      P2 (consumes src): b0: w = c - r; b1: m' = u + w -> out+a flops;
          b2: capture m' from PREV_ALU_OUT into delay lane 3 (one cycle
          after b1 wrote it, one cycle before P0(e+1) overwrites it);
          write DELAY_3 at stage 7.
    SRC_0 presents data only on require_inp0 slots (HW-verified), so c is
    read exactly on P2. Seed 2 uops; 8 drain slots flush the pipe."""
    from concourse.dve_uop import (
        ENABLE,
        AluInp,
        AluOp,
        DelayInp,
        InpSel,
        OutPath,
        OutSel,
        Trigger,
        UopConfig,
    )

    def mk(write=False, consume=False, trigger=None, nxt=None, repeat=1):
        u = UopConfig()
        u.enable_input(InpSel.SRC_0, 1)  # chain0: c (stream head)
        u.enable_input(InpSel.CONST_0, 2)  # chain1: beta
        u.enable_input(InpSel.ONE_F32, 3)  # chain2: threshold 1.0
        u.enable_input(InpSel.CONST_1, 5)  # chain4: initial m
        for b_i, b in enumerate(u.datapath_config):
            b.pass_through_delay(0, 1, 2, 4)
            if b_i >= 3:
                b.pass_through_delay(3)
        if write:
            u.enable_output(OutSel.DELAY_3, OutPath.WR0_LO)
        if consume:
            u.require_inp0 = ENABLE
        u.trigger = trigger
        u.next_uop = nxt
        u.repeat_count = repeat
        return u

    T_, N_ = Trigger, None
    CNT = T_.COUNT
    NONE = T_.NONE

    def cnt(nxt_i):
        return ((CNT, NONE, NONE), (nxt_i, 0, 0))

    uops = []
    # 0: seed A — let input lanes land in block0's delay flops
    tr, nx = cnt(1)
    uops.append(mk(trigger=tr, nxt=nx))
    # 1: seed B — block1.out/a <- initial m (CONST_1 via chain4)
    tr, nx = cnt(2)
    u = mk(trigger=tr, nxt=nx)
    u.datapath_config[1].enable_alu(AluOp.BYPASS, AluInp.PREV_DELAY_4)
    u.datapath_config[1].alu_out_a_enable = ENABLE
    uops.append(u)
    # 2: P0 — u = beta*m @ b1
    tr, nx = cnt(3)
    u = mk(trigger=tr, nxt=nx)
    u.datapath_config[1].enable_alu(
        AluOp.MULTIPLY, AluInp.CURR_ALU_OUT, AluInp.PREV_DELAY_1
    )
    uops.append(u)
    # 3: P1 — r = (m > 1) @ b0, reading block1's freshly-written a-flop
    tr, nx = cnt(4)
    u = mk(trigger=tr, nxt=nx)
    u.datapath_config[0].enable_alu(
        AluOp.IS_GT, AluInp.NEXT_ALU_OUT_A, AluInp.PREV_DELAY_2
    )
    uops.append(u)
    # 4: P2 — consume c; w = c - r @ b0; m' = u + w @ b1 (out + a flops);
    #    capture m' into chain 3 @ b2; emit DELAY_3 at stage 7
    u = mk(
        write=True,
        consume=True,
        trigger=(T_.SRC_TENSOR_DONE, CNT, NONE),
        nxt=(5, 2, 0),
    )
    u.datapath_config[0].enable_alu(
        AluOp.SUBTRACT, AluInp.PREV_DELAY_0, AluInp.CURR_ALU_OUT
    )
    u.datapath_config[1].enable_alu(
        AluOp.ADD, AluInp.CURR_ALU_OUT, AluInp.PREV_ALU_OUT
    )
    u.datapath_config[1].alu_out_a_enable = ENABLE
    u.datapath_config[2].enable_delay_from_src(DelayInp.PREV_ALU_OUT, 3)
    uops.append(u)
    # 5: drain — 8 flush slots so the last P2 slot reaches stage 7, then IDLE
    u = mk(trigger=(CNT, NONE, NONE), nxt=(0, 0, 0), repeat=8)
    uops.append(u)
    for u in uops:
        u.validate("v3")
    return uops


_build_lif_uops = _build_lif_uops_v2


def _register_lif_op():
    import numpy as np_

    import concourse.dve_ops as dve_ops
    from concourse.dve_ops import DveOp
    from concourse.dve_spec import Spec, Src0
    from concourse.dve_uop import DveOpSpec

    if LIF_OP_NAME in dve_ops._SUB_OPCODE_FOR_NAME:
        return _CACHE["lif_op"]

    def _ref(in0, in1, c0, c1, c2):
        out = np_.empty((in0.shape[0], in0.shape[1]), np_.float32)
        m = np_.full((in0.shape[0],), c1, np_.float32)
        for t in range(in0.shape[1]):
            r = (m > 1.0).astype(np_.float32)
            m = np_.float32(c0) * m + (in0[:, t] - r)
            out[:, t] = m
        return out

    class _RawDveOp(DveOp):
        def compile(self, ver):
            assert ver == "v3", "LIF_SCAN_ANT is v3/TRN2-only"
            return DveOpSpec(
                name=self.name,
                opcode=dve_ops.get_dve_sub_opcode(self.name),
                uops=_build_lif_uops(),
                rd1_en=False,
            )

    op = _RawDveOp(
        name=LIF_OP_NAME,
        spec=Spec(body=Src0, reference=_ref),
        subdim=False,
        uops_sha={},
    )
    dve_ops.OPS.append(op)
    dve_ops._SUB_OPCODE_FOR_NAME[op.name] = (
        dve_ops._CUSTOM_DVE_ROW_BASE + len(dve_ops.OPS) - 1
    )
    dve_ops.CUSTOM_DVE_SPECS[op.name] = op.spec
    _CACHE["lif_op"] = op
    return op


def _build_nc_fp16(tq: int):
    """fp16 single-pass matmul + custom LIF DVE scan, bf16 outputs.

    tq = time-chunk size (512 or 256). cur is accumulated per (o, chunk)
    in PSUM; each chunk's LIF scan chains its initial membrane state from
    the previous chunk via a per-partition scalar AP (s1).
    """
    import concourse.bacc as bacc
    import concourse.mybir as mybir
    from concourse.tile import TileContext

    F32 = mybir.dt.float32
    BF16 = mybir.dt.bfloat16
    FP16 = mybir.dt.float16
    Op = mybir.AluOpType
    lif_op = _register_lif_op()
    assert tq == 512, "per-(o,q) PSUM tiling needs OT*NQ == 8 banks (tq=512)"
    NQ = T // tq  # number of time chunks
    CW = tq  # scan-out chunk width (out len == src len)

    nc = bacc.Bacc(target_bir_lowering=False)
    # partition-major DRAM layout: each DMA chunk reads dense per-partition
    # runs instead of 1 KB runs strided across a 1 MB window
    xT_d = nc.dram_tensor("xT", [128, KT, T], FP16, kind="ExternalInput")
    WT_d = nc.dram_tensor("WT", [128, KT, O_SHARD], FP16, kind="ExternalInput")
    mem_d = nc.dram_tensor("mem", [O_SHARD, T], F32, kind="ExternalOutput")

    with TileContext(nc) as tc:
        with (
            tc.tile_pool(name="sb", bufs=1) as sb,
            tc.tile_pool(name="psp", bufs=1, space="PSUM") as psp,
        ):
            wt = sb.tile([128, KT, O_SHARD], FP16, name="wt")
            wt_view = WT_d
            xh = sb.tile([128, KT, T], FP16, name="xh")
            xh_view = xT_d
            # interleave W and first-chunk x k-wise so matmul k=0 can start
            # after the first two transfers; remaining x streams time-major
            # geometric first-chunk ramp: the 1-ktile pair completes its
            # sub-transfers (the matmul gate) sooner; later chunks grow
            for lo, hi in [(0, 1), (1, 2), (2, 4), (4, 8)] + [
                (kc, kc + 4) for kc in range(8, KT, 4)
            ]:
                nc.sync.dma_start(wt[:, lo:hi, :], wt_view[:, lo:hi, :])
                nc.sync.dma_start(xh[:, lo:hi, 0:tq], xh_view[:, lo:hi, 0:tq])
            # q>=1 x chunks follow immediately (they finish well before the
            # q1 matmuls start, protecting against a late-chunk stream gap)
            for q in range(1, NQ):
                tl, tr = q * tq, (q + 1) * tq
                for kc in range(0, KT, 8):
                    nc.sync.dma_start(
                        xh[:, kc : kc + 8, tl:tr], xh_view[:, kc : kc + 8, tl:tr]
                    )

            # one PSUM tile per (o, q) so a chunk's matmul group has no
            # write-after-read hazard against the previous chunk's scan
            ps = [
                [
                    psp.tile([128, tq], F32, name=f"ps{o}_{q}", tag=f"ps{o}_{q}")
                    for q in range(NQ)
                ]
                for o in range(OT)
            ]
            # the last (o, q) chunk is computed as two half-width groups in
            # two DISTINCT recycled q0 psum tiles (free after their q0 scans,
            # and distinct tiles avoid the tile-granular WAR hazard between
            # half-A's scan and half-B's matmuls)
            ps_last = [ps[0][0], ps[1][0]]
            M = sb.tile([128, OT, NQ * CW], F32, name="M")

            # HAM pre-warm: dummy matmuls on scratch data while the first
            # input chunks are still in flight, so the real stream starts at
            # the full 2.4 GHz clock and hides the PE queue's cold-dispatch
            # latency. Results land in ps[0][-1], which that tile's real
            # accumulation group's start=True later clears.
            scr = sb.tile([128, 256], FP16, name="scr")
            nc.vector.memset(scr, 0.0)
            for _ in range(38):
                nc.tensor.matmul(
                    ps[0][NQ - 1][:, 0:128],
                    lhsT=scr[:, 0:128],
                    rhs=scr[:, 128:256],
                    start=True,
                    stop=True,
                )

            def scan_epilogue(q, o):
                tl, tr = q * tq, (q + 1) * tq
                base = q * CW
                init = 0.0 if q == 0 else M[:, o, base - 1 : base]
                nc.vector._custom_dve(
                    lif_op,
                    out=M[:, o, base : base + CW],
                    in0=ps[o][q],
                    s0=BETA,
                    s1=init,
                )
                nc.sync.dma_start(
                    mem_d[o * 128 : (o + 1) * 128, tl:tr], M[:, o, base : base + CW]
                )

            for q in range(NQ):
                tl, tr = q * tq, (q + 1) * tq
                if q == 0:
                    # k-outer: matmuls track the streaming x DMA for q0
                    for k in range(KT):
                        for o in range(OT):
                            nc.tensor.matmul(
                                ps[o][q][:, :],
                                lhsT=wt[:, k, o * 128 : (o + 1) * 128],
                                rhs=xh[:, k, tl:tr],
                                start=(k == 0),
                                stop=(k == KT - 1),
                            )
                    for o in range(OT):
                        scan_epilogue(q, o)
                else:
                    # o-outer: each o-tile's PSUM completes early so its scan
                    # (Vector) overlaps the next o-tile's matmuls
                    for o in range(OT):
                        if not (q == NQ - 1 and o == OT - 1):
                            for k in range(KT):
                                nc.tensor.matmul(
                                    ps[o][q][:, :],
                                    lhsT=wt[:, k, o * 128 : (o + 1) * 128],
                                    rhs=xh[:, k, tl:tr],
                                    start=(k == 0),
                                    stop=(k == KT - 1),
                                )
                            scan_epilogue(q, o)
                        else:
                            # very last o-tile: two half-width accumulation
                            # groups so its first scan overlaps the second
                            # group's matmuls, halving the serial tail
                            for half, (cl, cr) in enumerate([(0, 3 * tq // 4), (3 * tq // 4, tq)]):
                                h = cr - cl
                                for k in range(KT):
                                    nc.tensor.matmul(
                                        ps_last[half][:, 0:h],
                                        lhsT=wt[:, k, o * 128 : (o + 1) * 128],
                                        rhs=xh[:, k, tl + cl : tl + cr],
                                        start=(k == 0),
                                        stop=(k == KT - 1),
                                    )
                                base = q * CW + cl
                                nc.vector._custom_dve(
                                    lif_op,
                                    out=M[:, o, base : base + h],
                                    in0=ps_last[half][:, 0:h],
                                    s0=BETA,
                                    s1=M[:, o, base - 1 : base],
                                )
                                nc.sync.dma_start(
                                    mem_d[o * 128 : (o + 1) * 128, tl + cl : tl + cr],
                                    M[:, o, base : base + h],
                                )
    nc.finalize()
    return nc


def _build_nc(mm_dtype_name: str):
    import concourse.bacc as bacc
    import concourse.mybir as mybir
    from concourse.tile import TileContext

    if mm_dtype_name.startswith("fp16"):
        return _build_nc_fp16(int(mm_dtype_name[4:] or 512))

    F32 = mybir.dt.float32
    custom = mm_dtype_name == "custom"
    split3 = mm_dtype_name == "split3" or custom
    MMDT = mybir.dt.bfloat16 if split3 else getattr(mybir.dt, mm_dtype_name)
    Op = mybir.AluOpType
    lif_op = _register_lif_op() if custom else None

    nc = bacc.Bacc(target_bir_lowering=False)
    # split3: xT/WT carry [2, ...] leading dim = (hi, lo) bf16 parts.
    xshape = [2, N_IN, T] if split3 else [N_IN, T]
    wshape = [2, N_IN, O_SHARD] if split3 else [N_IN, O_SHARD]
    xT_d = nc.dram_tensor("xT", xshape, MMDT, kind="ExternalInput")
    WT_d = nc.dram_tensor("WT", wshape, MMDT, kind="ExternalInput")
    spk_d = nc.dram_tensor("spk", [O_SHARD, T], F32, kind="ExternalOutput")
    mem_d = nc.dram_tensor("mem", [O_SHARD, T], F32, kind="ExternalOutput")

    with TileContext(nc) as tc:
        with (
            tc.tile_pool(name="sb", bufs=1) as sb,
            tc.tile_pool(name="xs", bufs=4) as xs,
            tc.tile_pool(name="psp", bufs=1, space="PSUM") as psp,
        ):
            # All weights resident: [128, 2|1, KT, O_SHARD]; k-tile k holds
            # WT rows k*128..k*128+127 (i.e. W.T), so wt[:, h, k, o*128:...]
            # is directly the matmul stationary operand [K=128, M=128].
            NH = 2 if split3 else 1
            wt = sb.tile([128, NH, KT, O_SHARD], MMDT, name="wt")
            if split3:
                wt_view = WT_d.rearrange("h (k p) o -> p h k o", p=128)
            else:
                wt_view = WT_d.rearrange("(k p) o -> p () k o", p=128)
            for h in range(NH):
                for kc in range(0, KT, 4):
                    nc.sync.dma_start(
                        wt[:, h, kc : kc + 4, :], wt_view[:, h, kc : kc + 4, :]
                    )

            ps = [
                psp.tile([128, T], F32, name=f"ps{o}", tag=f"ps{o}") for o in range(OT)
            ]

            if custom:
                M = sb.tile([128, OT, T], F32, name="M")
                Sp = sb.tile([128, OT, T], F32, name="Sp")
            else:
                A = sb.tile([128, OT, T], F32, name="A")  # linear-part scan
                TH = sb.tile([128, OT, T], F32, name="TH")  # theta = 1 - A
                M = sb.tile([128, OT, T], F32, name="M")  # mem = A + B
                Bb = sb.tile([128, OT, T + 1], F32, name="Bb")  # residual state
                Sb = sb.tile([128, OT, T + 1], F32, name="Sb")  # spikes (0/1)
                beta_t = sb.tile([128, 512], F32, name="beta_t")

                nc.vector.memset(beta_t, BETA)
                nc.vector.memset(Bb[:, :, 0], 0.0)
                nc.vector.memset(Sb[:, :, 0], 0.0)

            for th in range(2):
                tl, tr = th * 512, (th + 1) * 512
                # ---- matmul: accumulate cur[:, tl:tr] over all K ----
                for k in range(KT):
                    if split3:
                        xh = xs.tile([128, 2, 512], MMDT, name="xh")
                        nc.sync.dma_start(
                            xh,
                            xT_d[:, k * 128 : (k + 1) * 128, tl:tr].rearrange(
                                "h p t -> p h t"
                            ),
                        )
                        terms = [(0, 0), (1, 0), (0, 1)]  # (h_w, h_x)
                    else:
                        xh = xs.tile([128, 1, 512], MMDT, name="xh")
                        nc.sync.dma_start(
                            xh[:, 0, :], xT_d[k * 128 : (k + 1) * 128, tl:tr]
                        )
                        terms = [(0, 0)]
                    for o in range(OT):
                        for ti, (hw, hx) in enumerate(terms):
                            nc.tensor.matmul(
                                ps[o][:, tl:tr],
                                lhsT=wt[:, hw, k, o * 128 : (o + 1) * 128],
                                rhs=xh[:, hx, :],
                                start=(k == 0 and ti == 0),
                                stop=(k == KT - 1 and ti == len(terms) - 1),
                            )
                if custom:
                    for o in range(OT):
                        base = th * 512
                        init = 0.0 if th == 0 else M[:, o, 511:512]
                        nc.vector._custom_dve(
                            lif_op,
                            out=M[:, o, base : base + 512],
                            in0=ps[o][:, tl:tr],
                            s0=BETA,
                            s1=init,
                        )
                        nc.vector.tensor_scalar(
                            Sp[:, o, tl:tr],
                            M[:, o, base : base + 512],
                            1.0,
                            None,
                            Op.is_gt,
                        )
                        nc.sync.dma_start(
                            spk_d[o * 128 : (o + 1) * 128, tl:tr], Sp[:, o, tl:tr]
                        )
                        nc.sync.dma_start(
                            mem_d[o * 128 : (o + 1) * 128, tl:tr],
                            M[:, o, base : base + 512],
                        )
                    continue
                # ---- bulk prep for this half: A scan + theta ----
                for o in range(OT):
                    init = 0.0 if th == 0 else A[:, o, tl - 1 : tl]
                    nc.vector.tensor_tensor_scan(
                        out=A[:, o, tl:tr],
                        data0=beta_t,
                        data1=ps[o][:, tl:tr],
                        initial=init,
                        op0=Op.mult,
                        op1=Op.add,
                    )
                    nc.gpsimd.tensor_scalar(
                        TH[:, o, tl:tr], A[:, o, tl:tr], -1.0, THRESHOLD, Op.mult, Op.add
                    )
                # ---- serial scan for this half: 2 DVE instrs per step ----
                for t in range(tl + 1, tr + 1):
                    nc.vector.scalar_tensor_tensor(
                        out=Bb[:, :, t],
                        in0=Bb[:, :, t - 1],
                        scalar=BETA,
                        in1=Sb[:, :, t - 1],
                        op0=Op.mult,
                        op1=Op.subtract,
                    )
                    nc.vector.tensor_tensor(
                        Sb[:, :, t], Bb[:, :, t], TH[:, :, t - 1], Op.is_gt
                    )
                # ---- epilogue for this half: mem = A + B, DMA out ----
                for o in range(OT):
                    nc.gpsimd.tensor_tensor(
                        M[:, o, tl:tr], A[:, o, tl:tr], Bb[:, o, tl + 1 : tr + 1], Op.add
                    )
                    nc.sync.dma_start(
                        spk_d[o * 128 : (o + 1) * 128, tl:tr], Sb[:, o, tl + 1 : tr + 1]
                    )
                    nc.sync.dma_start(
                        mem_d[o * 128 : (o + 1) * 128, tl:tr], M[:, o, tl:tr]
                    )
    nc.finalize()
    return nc


def _get_nc(mm_dtype_name: str):
    if mm_dtype_name not in _CACHE:
        _CACHE[mm_dtype_name] = _build_nc(mm_dtype_name)
    return _CACHE[mm_dtype_name]


def run(x, W, mm_dtype_name="split3", trace=False):
    import ml_dtypes

    from concourse.bass_utils import run_bass_kernel_spmd

    bf16 = ml_dtypes.bfloat16
    nc = _get_nc(mm_dtype_name)
    x = np.asarray(x, dtype=np.float32)
    W = np.asarray(W, dtype=np.float32)
    in_maps = []
    if mm_dtype_name.startswith("fp16"):
        # partition-major [128, KT, *] layouts (see _build_nc_fp16)
        xT = np.ascontiguousarray(
            x.T.astype(np.float16).reshape(KT, 128, T).transpose(1, 0, 2)
        )
        for c in range(N_CORES):
            WTc = np.ascontiguousarray(
                W[c * O_SHARD : (c + 1) * O_SHARD, :]
                .T.astype(np.float16)
                .reshape(KT, 128, O_SHARD)
                .transpose(1, 0, 2)
            )
            in_maps.append({"xT": xT, "WT": WTc})
        res = run_bass_kernel_spmd(
            nc, in_maps, core_ids=list(range(N_CORES)), trace=trace
        )
        mem = np.ascontiguousarray(
            np.concatenate(
                [np.asarray(r["mem"], dtype=np.float32) for r in res.results], axis=0
            ).T
        )
        # spike = (mem > 1) computed host-side; bit-identical to the device
        # is_gt on the same f32 mem values
        spk = (mem > np.float32(THRESHOLD)).astype(np.float32)
        return (spk, mem), res
    if mm_dtype_name == "split3":
        x_hi = x.astype(bf16)
        x_lo = (x - x_hi.astype(np.float32)).astype(bf16)
        xT = np.ascontiguousarray(
            np.stack([x_hi.T, x_lo.T], axis=0)
        )  # [2, N_IN, T] bf16
        W_hi = W.astype(bf16)
        W_lo = (W - W_hi.astype(np.float32)).astype(bf16)
        for c in range(N_CORES):
            sl = slice(c * O_SHARD, (c + 1) * O_SHARD)
            WTc = np.ascontiguousarray(np.stack([W_hi[sl].T, W_lo[sl].T], axis=0))
            in_maps.append({"xT": xT, "WT": WTc})
    else:
        xT = np.ascontiguousarray(x.T)  # [N_IN, T]
        for c in range(N_CORES):
            WTc = np.ascontiguousarray(W[c * O_SHARD : (c + 1) * O_SHARD, :].T)
            in_maps.append({"xT": xT, "WT": WTc})
    res = run_bass_kernel_spmd(nc, in_maps, core_ids=list(range(N_CORES)), trace=trace)
    spk = np.concatenate([r["spk"] for r in res.results], axis=0).T
    mem = np.concatenate([r["mem"] for r in res.results], axis=0).T
    return (
        np.ascontiguousarray(spk),
        np.ascontiguousarray(mem),
    ), res


def kernel(x, W):
    out, _ = run(x, W, mm_dtype_name="fp16512")
    return out



# revision 52
# speedup vs baseline: 1.0304x; 1.0304x over previous
"""Trainium2 Bass kernel: SNN Leaky-Integrate-and-Fire layer.

Computes, for x [T=1024, N_IN=4096] f32 and W [N_OUT=4096, N_IN=4096] f32:
    cur = x @ W.T                                   # [T, N_OUT]
    mem_t = 0.9*mem_{t-1} + cur_t - (mem_{t-1} > 1)  # scan over T
    spk_t = (mem_t > 1)
returning (spk_rec, mem_rec), both [T, N_OUT] f32.

Sharding: N_OUT split across 8 NeuronCores (512 neurons each); x is
replicated. No cross-core communication.

Default device algorithm per core ("fp16512", neurons on partitions,
time along the free dim):
  - single-pass fp16 matmul: x and W are cast to fp16 host-side
    (products are exact in the fp32 PSUM accumulate; quantization gives
    ~508/4.2M spike flips, rel err 1.7e-2 < 2e-2). 256 matmuls:
    K=4096 in 32 tiles x O=512 in 4 o-tiles x T=1024 in 2 chunks of 512,
    each (o, chunk) accumulating into its own PSUM bank.
  - the full nonlinear LIF scan runs in ONE custom DVE instruction per
    (o, chunk) [LIF_SCAN_ANT, 3 cycles/element, see _build_lif_uops_v2],
    chaining membrane state across chunks via a per-partition scalar AP.
  - mem is DMA'd out in f32; spikes are computed host-side as mem > 1
    (bit-identical to an on-device is_gt on the same f32 values).
Older variants kept for comparison: "split3" (3-pass bf16 hi/lo matmul +
A/B-decomposed scan, 2 serial DVE instrs per timestep) and "custom"
(split3 matmul + custom DVE scan).
"""

import numpy as np

T = 1024
N_IN = 4096
N_OUT = 4096
N_CORES = 8
O_SHARD = N_OUT // N_CORES  # 512
KT = N_IN // 128  # 32 k-tiles
OT = O_SHARD // 128  # 4 o-tiles
BETA = 0.9
THRESHOLD = 1.0

_CACHE = {}

LIF_OP_NAME = "LIF_SCAN_ANT"


def _build_lif_uops_v2():
    """Hand-written DVE uop program for the LIF recurrence:

        m' = beta*m + c[i] - (m > 1)    (beta = CONST_0/s0, init m = CONST_1/s1)
        out[i] = m'   (out len == src len, 3 cycles per element)

    State m lives in block1's out-flop (+ a-flop for the block0 backward
    read). Per element e with P0/P1/P2 entering at consecutive cycles:
      P0: b1: u = beta*m        (CURR_ALU_OUT x PREV_DELAY_1)
      P1: b0: r = IS_GT(m, 1)   (NEXT_ALU_OUT_A vs PREV_DELAY_2) - reads the
          a-flop one cycle after P2(e-1) wrote m', so it is fresh
      P2 (consumes src): b0: w = c - r; b1: m' = u + w -> out+a flops;
          b2: capture m' from PREV_ALU_OUT into delay lane 3 (one cycle
          after b1 wrote it, one cycle before P0(e+1) overwrites it);
          write DELAY_3 at stage 7.
    SRC_0 presents data only on require_inp0 slots (HW-verified), so c is
    read exactly on P2. Seed 2 uops; 8 drain slots flush the pipe."""
    from concourse.dve_uop import (
        ENABLE,
        AluInp,
        AluOp,
        DelayInp,
        InpSel,
        OutPath,
        OutSel,
        Trigger,
        UopConfig,
    )

    def mk(write=False, consume=False, trigger=None, nxt=None, repeat=1):
        u = UopConfig()
        u.enable_input(InpSel.SRC_0, 1)  # chain0: c (stream head)
        u.enable_input(InpSel.CONST_0, 2)  # chain1: beta
        u.enable_input(InpSel.ONE_F32, 3)  # chain2: threshold 1.0
        u.enable_input(InpSel.CONST_1, 5)  # chain4: initial m
        for b_i, b in enumerate(u.datapath_config):
            b.pass_through_delay(0, 1, 2, 4)
            if b_i >= 3:
                b.pass_through_delay(3)
        if write:
            u.enable_output(OutSel.DELAY_3, OutPath.WR0_LO)
        if consume:
            u.require_inp0 = ENABLE
        u.trigger = trigger
        u.next_uop = nxt
        u.repeat_count = repeat
        return u

    T_, N_ = Trigger, None
    CNT = T_.COUNT
    NONE = T_.NONE

    def cnt(nxt_i):
        return ((CNT, NONE, NONE), (nxt_i, 0, 0))

    uops = []
    # 0: seed A — let input lanes land in block0's delay flops
    tr, nx = cnt(1)
    uops.append(mk(trigger=tr, nxt=nx))
    # 1: seed B — block1.out/a <- initial m (CONST_1 via chain4)
    tr, nx = cnt(2)
    u = mk(trigger=tr, nxt=nx)
    u.datapath_config[1].enable_alu(AluOp.BYPASS, AluInp.PREV_DELAY_4)
    u.datapath_config[1].alu_out_a_enable = ENABLE
    uops.append(u)
    # 2: P0 — u = beta*m @ b1
    tr, nx = cnt(3)
    u = mk(trigger=tr, nxt=nx)
    u.datapath_config[1].enable_alu(
        AluOp.MULTIPLY, AluInp.CURR_ALU_OUT, AluInp.PREV_DELAY_1
    )
    uops.append(u)
    # 3: P1 — r = (m > 1) @ b0, reading block1's freshly-written a-flop
    tr, nx = cnt(4)
    u = mk(trigger=tr, nxt=nx)
    u.datapath_config[0].enable_alu(
        AluOp.IS_GT, AluInp.NEXT_ALU_OUT_A, AluInp.PREV_DELAY_2
    )
    uops.append(u)
    # 4: P2 — consume c; w = c - r @ b0; m' = u + w @ b1 (out + a flops);
    #    capture m' into chain 3 @ b2; emit DELAY_3 at stage 7
    u = mk(
        write=True,
        consume=True,
        trigger=(T_.SRC_TENSOR_DONE, CNT, NONE),
        nxt=(5, 2, 0),
    )
    u.datapath_config[0].enable_alu(
        AluOp.SUBTRACT, AluInp.PREV_DELAY_0, AluInp.CURR_ALU_OUT
    )
    u.datapath_config[1].enable_alu(
        AluOp.ADD, AluInp.CURR_ALU_OUT, AluInp.PREV_ALU_OUT
    )
    u.datapath_config[1].alu_out_a_enable = ENABLE
    u.datapath_config[2].enable_delay_from_src(DelayInp.PREV_ALU_OUT, 3)
    uops.append(u)
    # 5: drain — 8 flush slots so the last P2 slot reaches stage 7, then IDLE
    u = mk(trigger=(CNT, NONE, NONE), nxt=(0, 0, 0), repeat=8)
    uops.append(u)
    for u in uops:
        u.validate("v3")
    return uops


_build_lif_uops = _build_lif_uops_v2


def _register_lif_op():
    import numpy as np_

    import concourse.dve_ops as dve_ops
    from concourse.dve_ops import DveOp
    from concourse.dve_spec import Spec, Src0
    from concourse.dve_uop import DveOpSpec

    if LIF_OP_NAME in dve_ops._SUB_OPCODE_FOR_NAME:
        return _CACHE["lif_op"]

    def _ref(in0, in1, c0, c1, c2):
        out = np_.empty((in0.shape[0], in0.shape[1]), np_.float32)
        m = np_.full((in0.shape[0],), c1, np_.float32)
        for t in range(in0.shape[1]):
            r = (m > 1.0).astype(np_.float32)
            m = np_.float32(c0) * m + (in0[:, t] - r)
            out[:, t] = m
        return out

    class _RawDveOp(DveOp):
        def compile(self, ver):
            assert ver == "v3", "LIF_SCAN_ANT is v3/TRN2-only"
            return DveOpSpec(
                name=self.name,
                opcode=dve_ops.get_dve_sub_opcode(self.name),
                uops=_build_lif_uops(),
                rd1_en=False,
            )

    op = _RawDveOp(
        name=LIF_OP_NAME,
        spec=Spec(body=Src0, reference=_ref),
        subdim=False,
        uops_sha={},
    )
    dve_ops.OPS.append(op)
    dve_ops._SUB_OPCODE_FOR_NAME[op.name] = (
        dve_ops._CUSTOM_DVE_ROW_BASE + len(dve_ops.OPS) - 1
    )
    dve_ops.CUSTOM_DVE_SPECS[op.name] = op.spec
    _CACHE["lif_op"] = op
    return op


def _build_nc_fp16(tq: int):
    """fp16 single-pass matmul + custom LIF DVE scan, bf16 outputs.

    tq = time-chunk size (512 or 256). cur is accumulated per (o, chunk)
    in PSUM; each chunk's LIF scan chains its initial membrane state from
    the previous chunk via a per-partition scalar AP (s1).
    """
    import concourse.bacc as bacc
    import concourse.mybir as mybir
    from concourse.tile import TileContext

    F32 = mybir.dt.float32
    BF16 = mybir.dt.bfloat16
    FP16 = mybir.dt.float16
    Op = mybir.AluOpType
    lif_op = _register_lif_op()
    assert tq == 512, "per-(o,q) PSUM tiling needs OT*NQ == 8 banks (tq=512)"
    NQ = T // tq  # number of time chunks
    CW = tq  # scan-out chunk width (out len == src len)

    nc = bacc.Bacc(target_bir_lowering=False)
    # partition-major DRAM layout: each DMA chunk reads dense per-partition
    # runs instead of 1 KB runs strided across a 1 MB window
    xT_d = nc.dram_tensor("xT", [128, KT, T], FP16, kind="ExternalInput")
    WT_d = nc.dram_tensor("WT", [128, KT, O_SHARD], FP16, kind="ExternalInput")
    mem_d = nc.dram_tensor("mem", [O_SHARD, T], F32, kind="ExternalOutput")

    with TileContext(nc) as tc:
        with (
            tc.tile_pool(name="sb", bufs=1) as sb,
            tc.tile_pool(name="psp", bufs=1, space="PSUM") as psp,
        ):
            wt = sb.tile([128, KT, O_SHARD], FP16, name="wt")
            wt_view = WT_d
            xh = sb.tile([128, KT, T], FP16, name="xh")
            xh_view = xT_d
            # interleave W and first-chunk x k-wise so matmul k=0 can start
            # after the first two transfers; remaining x streams time-major
            for kc in range(0, KT, 4):
                nc.sync.dma_start(wt[:, kc : kc + 4, :], wt_view[:, kc : kc + 4, :])
                nc.sync.dma_start(
                    xh[:, kc : kc + 4, 0:tq], xh_view[:, kc : kc + 4, 0:tq]
                )
            # q>=1 x chunks follow immediately (they finish well before the
            # q1 matmuls start, protecting against a late-chunk stream gap)
            for q in range(1, NQ):
                tl, tr = q * tq, (q + 1) * tq
                for kc in range(0, KT, 8):
                    nc.sync.dma_start(
                        xh[:, kc : kc + 8, tl:tr], xh_view[:, kc : kc + 8, tl:tr]
                    )

            # one PSUM tile per (o, q) so a chunk's matmul group has no
            # write-after-read hazard against the previous chunk's scan
            ps = [
                [
                    psp.tile([128, tq], F32, name=f"ps{o}_{q}", tag=f"ps{o}_{q}")
                    for q in range(NQ)
                ]
                for o in range(OT)
            ]
            # the last (o, q) chunk is computed as two half-width groups in
            # two DISTINCT recycled q0 psum tiles (free after their q0 scans,
            # and distinct tiles avoid the tile-granular WAR hazard between
            # half-A's scan and half-B's matmuls)
            ps_last = [ps[0][0], ps[1][0]]
            M = sb.tile([128, OT, NQ * CW], F32, name="M")

            # HAM pre-warm: dummy matmuls on scratch data while the first
            # input chunks are still in flight, so the real stream starts at
            # the full 2.4 GHz clock and hides the PE queue's cold-dispatch
            # latency. Results land in ps[0][-1], which that tile's real
            # accumulation group's start=True later clears.
            scr = sb.tile([128, 256], FP16, name="scr")
            nc.vector.memset(scr, 0.0)
            for _ in range(52):
                nc.tensor.matmul(
                    ps[0][NQ - 1][:, 0:128],
                    lhsT=scr[:, 0:128],
                    rhs=scr[:, 128:256],
                    start=True,
                    stop=True,
                )

            def scan_epilogue(q, o):
                tl, tr = q * tq, (q + 1) * tq
                base = q * CW
                init = 0.0 if q == 0 else M[:, o, base - 1 : base]
                nc.vector._custom_dve(
                    lif_op,
                    out=M[:, o, base : base + CW],
                    in0=ps[o][q],
                    s0=BETA,
                    s1=init,
                )
                nc.sync.dma_start(
                    mem_d[o * 128 : (o + 1) * 128, tl:tr], M[:, o, base : base + CW]
                )

            for q in range(NQ):
                tl, tr = q * tq, (q + 1) * tq
                if q == 0:
                    # k-outer: matmuls track the streaming x DMA for q0
                    for k in range(KT):
                        for o in range(OT):
                            nc.tensor.matmul(
                                ps[o][q][:, :],
                                lhsT=wt[:, k, o * 128 : (o + 1) * 128],
                                rhs=xh[:, k, tl:tr],
                                start=(k == 0),
                                stop=(k == KT - 1),
                            )
                    for o in range(OT):
                        scan_epilogue(q, o)
                else:
                    # o-outer: each o-tile's PSUM completes early so its scan
                    # (Vector) overlaps the next o-tile's matmuls
                    for o in range(OT):
                        if not (q == NQ - 1 and o == OT - 1):
                            for k in range(KT):
                                nc.tensor.matmul(
                                    ps[o][q][:, :],
                                    lhsT=wt[:, k, o * 128 : (o + 1) * 128],
                                    rhs=xh[:, k, tl:tr],
                                    start=(k == 0),
                                    stop=(k == KT - 1),
                                )
                            scan_epilogue(q, o)
                        else:
                            # very last o-tile: two half-width accumulation
                            # groups so its first scan overlaps the second
                            # group's matmuls, halving the serial tail
                            for half, (cl, cr) in enumerate([(0, 3 * tq // 4), (3 * tq // 4, tq)]):
                                h = cr - cl
                                for k in range(KT):
                                    nc.tensor.matmul(
                                        ps_last[half][:, 0:h],
                                        lhsT=wt[:, k, o * 128 : (o + 1) * 128],
                                        rhs=xh[:, k, tl + cl : tl + cr],
                                        start=(k == 0),
                                        stop=(k == KT - 1),
                                    )
                                base = q * CW + cl
                                nc.vector._custom_dve(
                                    lif_op,
                                    out=M[:, o, base : base + h],
                                    in0=ps_last[half][:, 0:h],
                                    s0=BETA,
                                    s1=M[:, o, base - 1 : base],
                                )
                                nc.sync.dma_start(
                                    mem_d[o * 128 : (o + 1) * 128, tl + cl : tl + cr],
                                    M[:, o, base : base + h],
                                )
    nc.finalize()
    return nc


def _build_nc(mm_dtype_name: str):
    import concourse.bacc as bacc
    import concourse.mybir as mybir
    from concourse.tile import TileContext

    if mm_dtype_name.startswith("fp16"):
        return _build_nc_fp16(int(mm_dtype_name[4:] or 512))

    F32 = mybir.dt.float32
    custom = mm_dtype_name == "custom"
    split3 = mm_dtype_name == "split3" or custom
    MMDT = mybir.dt.bfloat16 if split3 else getattr(mybir.dt, mm_dtype_name)
    Op = mybir.AluOpType
    lif_op = _register_lif_op() if custom else None

    nc = bacc.Bacc(target_bir_lowering=False)
    # split3: xT/WT carry [2, ...] leading dim = (hi, lo) bf16 parts.
    xshape = [2, N_IN, T] if split3 else [N_IN, T]
    wshape = [2, N_IN, O_SHARD] if split3 else [N_IN, O_SHARD]
    xT_d = nc.dram_tensor("xT", xshape, MMDT, kind="ExternalInput")
    WT_d = nc.dram_tensor("WT", wshape, MMDT, kind="ExternalInput")
    spk_d = nc.dram_tensor("spk", [O_SHARD, T], F32, kind="ExternalOutput")
    mem_d = nc.dram_tensor("mem", [O_SHARD, T], F32, kind="ExternalOutput")

    with TileContext(nc) as tc:
        with (
            tc.tile_pool(name="sb", bufs=1) as sb,
            tc.tile_pool(name="xs", bufs=4) as xs,
            tc.tile_pool(name="psp", bufs=1, space="PSUM") as psp,
        ):
            # All weights resident: [128, 2|1, KT, O_SHARD]; k-tile k holds
            # WT rows k*128..k*128+127 (i.e. W.T), so wt[:, h, k, o*128:...]
            # is directly the matmul stationary operand [K=128, M=128].
            NH = 2 if split3 else 1
            wt = sb.tile([128, NH, KT, O_SHARD], MMDT, name="wt")
            if split3:
                wt_view = WT_d.rearrange("h (k p) o -> p h k o", p=128)
            else:
                wt_view = WT_d.rearrange("(k p) o -> p () k o", p=128)
            for h in range(NH):
                for kc in range(0, KT, 4):
                    nc.sync.dma_start(
                        wt[:, h, kc : kc + 4, :], wt_view[:, h, kc : kc + 4, :]
                    )

            ps = [
                psp.tile([128, T], F32, name=f"ps{o}", tag=f"ps{o}") for o in range(OT)
            ]

            if custom:
                M = sb.tile([128, OT, T], F32, name="M")
                Sp = sb.tile([128, OT, T], F32, name="Sp")
            else:
                A = sb.tile([128, OT, T], F32, name="A")  # linear-part scan
                TH = sb.tile([128, OT, T], F32, name="TH")  # theta = 1 - A
                M = sb.tile([128, OT, T], F32, name="M")  # mem = A + B
                Bb = sb.tile([128, OT, T + 1], F32, name="Bb")  # residual state
                Sb = sb.tile([128, OT, T + 1], F32, name="Sb")  # spikes (0/1)
                beta_t = sb.tile([128, 512], F32, name="beta_t")

                nc.vector.memset(beta_t, BETA)
                nc.vector.memset(Bb[:, :, 0], 0.0)
                nc.vector.memset(Sb[:, :, 0], 0.0)

            for th in range(2):
                tl, tr = th * 512, (th + 1) * 512
                # ---- matmul: accumulate cur[:, tl:tr] over all K ----
                for k in range(KT):
                    if split3:
                        xh = xs.tile([128, 2, 512], MMDT, name="xh")
                        nc.sync.dma_start(
                            xh,
                            xT_d[:, k * 128 : (k + 1) * 128, tl:tr].rearrange(
                                "h p t -> p h t"
                            ),
                        )
                        terms = [(0, 0), (1, 0), (0, 1)]  # (h_w, h_x)
                    else:
                        xh = xs.tile([128, 1, 512], MMDT, name="xh")
                        nc.sync.dma_start(
                            xh[:, 0, :], xT_d[k * 128 : (k + 1) * 128, tl:tr]
                        )
                        terms = [(0, 0)]
                    for o in range(OT):
                        for ti, (hw, hx) in enumerate(terms):
                            nc.tensor.matmul(
                                ps[o][:, tl:tr],
                                lhsT=wt[:, hw, k, o * 128 : (o + 1) * 128],
                                rhs=xh[:, hx, :],
                                start=(k == 0 and ti == 0),
                                stop=(k == KT - 1 and ti == len(terms) - 1),
                            )
                if custom:
                    for o in range(OT):
                        base = th * 512
                        init = 0.0 if th == 0 else M[:, o, 511:512]
                        nc.vector._custom_dve(
                            lif_op,
                            out=M[:, o, base : base + 512],
                            in0=ps[o][:, tl:tr],
                            s0=BETA,
                            s1=init,
                        )
                        nc.vector.tensor_scalar(
                            Sp[:, o, tl:tr],
                            M[:, o, base : base + 512],
                            1.0,
                            None,
                            Op.is_gt,
                        )
                        nc.sync.dma_start(
                            spk_d[o * 128 : (o + 1) * 128, tl:tr], Sp[:, o, tl:tr]
                        )
                        nc.sync.dma_start(
                            mem_d[o * 128 : (o + 1) * 128, tl:tr],
                            M[:, o, base : base + 512],
                        )
                    continue
                # ---- bulk prep for this half: A scan + theta ----
                for o in range(OT):
                    init = 0.0 if th == 0 else A[:, o, tl - 1 : tl]
                    nc.vector.tensor_tensor_scan(
                        out=A[:, o, tl:tr],
                        data0=beta_t,
                        data1=ps[o][:, tl:tr],
                        initial=init,
                        op0=Op.mult,
                        op1=Op.add,
                    )
                    nc.gpsimd.tensor_scalar(
                        TH[:, o, tl:tr], A[:, o, tl:tr], -1.0, THRESHOLD, Op.mult, Op.add
                    )
                # ---- serial scan for this half: 2 DVE instrs per step ----
                for t in range(tl + 1, tr + 1):
                    nc.vector.scalar_tensor_tensor(
                        out=Bb[:, :, t],
                        in0=Bb[:, :, t - 1],
                        scalar=BETA,
                        in1=Sb[:, :, t - 1],
                        op0=Op.mult,
                        op1=Op.subtract,
                    )
                    nc.vector.tensor_tensor(
                        Sb[:, :, t], Bb[:, :, t], TH[:, :, t - 1], Op.is_gt
                    )
                # ---- epilogue for this half: mem = A + B, DMA out ----
                for o in range(OT):
                    nc.gpsimd.tensor_tensor(
                        M[:, o, tl:tr], A[:, o, tl:tr], Bb[:, o, tl + 1 : tr + 1], Op.add
                    )
                    nc.sync.dma_start(
                        spk_d[o * 128 : (o + 1) * 128, tl:tr], Sb[:, o, tl + 1 : tr + 1]
                    )
                    nc.sync.dma_start(
                        mem_d[o * 128 : (o + 1) * 128, tl:tr], M[:, o, tl:tr]
                    )
    nc.finalize()
    return nc


def _get_nc(mm_dtype_name: str):
    if mm_dtype_name not in _CACHE:
        _CACHE[mm_dtype_name] = _build_nc(mm_dtype_name)
    return _CACHE[mm_dtype_name]


def run(x, W, mm_dtype_name="split3", trace=False):
    import ml_dtypes

    from concourse.bass_utils import run_bass_kernel_spmd

    bf16 = ml_dtypes.bfloat16
    nc = _get_nc(mm_dtype_name)
    x = np.asarray(x, dtype=np.float32)
    W = np.asarray(W, dtype=np.float32)
    in_maps = []
    if mm_dtype_name.startswith("fp16"):
        # partition-major [128, KT, *] layouts (see _build_nc_fp16)
        xT = np.ascontiguousarray(
            x.T.astype(np.float16).reshape(KT, 128, T).transpose(1, 0, 2)
        )
        for c in range(N_CORES):
            WTc = np.ascontiguousarray(
                W[c * O_SHARD : (c + 1) * O_SHARD, :]
                .T.astype(np.float16)
                .reshape(KT, 128, O_SHARD)
                .transpose(1, 0, 2)
            )
            in_maps.append({"xT": xT, "WT": WTc})
        res = run_bass_kernel_spmd(
            nc, in_maps, core_ids=list(range(N_CORES)), trace=trace
        )
        mem = np.ascontiguousarray(
            np.concatenate(
                [np.asarray(r["mem"], dtype=np.float32) for r in res.results], axis=0
            ).T
        )
        # spike = (mem > 1) computed host-side; bit-identical to the device
        # is_gt on the same f32 mem values
        spk = (mem > np.float32(THRESHOLD)).astype(np.float32)
        return (spk, mem), res
    if mm_dtype_name == "split3":
        x_hi = x.astype(bf16)
        x_lo = (x - x_hi.astype(np.float32)).astype(bf16)
        xT = np.ascontiguousarray(
            np.stack([x_hi.T, x_lo.T], axis=0)
        )  # [2, N_IN, T] bf16
        W_hi = W.astype(bf16)
        W_lo = (W - W_hi.astype(np.float32)).astype(bf16)
        for c in range(N_CORES):
            sl = slice(c * O_SHARD, (c + 1) * O_SHARD)
            WTc = np.ascontiguousarray(np.stack([W_hi[sl].T, W_lo[sl].T], axis=0))
            in_maps.append({"xT": xT, "WT": WTc})
    else:
        xT = np.ascontiguousarray(x.T)  # [N_IN, T]
        for c in range(N_CORES):
            WTc = np.ascontiguousarray(W[c * O_SHARD : (c + 1) * O_SHARD, :].T)
            in_maps.append({"xT": xT, "WT": WTc})
    res = run_bass_kernel_spmd(nc, in_maps, core_ids=list(range(N_CORES)), trace=trace)
    spk = np.concatenate([r["spk"] for r in res.results], axis=0).T
    mem = np.concatenate([r["mem"] for r in res.results], axis=0).T
    return (
        np.ascontiguousarray(spk),
        np.ascontiguousarray(mem),
    ), res


def kernel(x, W):
    out, _ = run(x, W, mm_dtype_name="fp16512")
    return out

